# revision 28
# baseline (speedup 1.0000x reference)
"""BGFusionBlock Trainium2 kernel (8 NeuronCores, spatial H-sharding).

Contract: kernel(**inputs) -> np.ndarray. Full inputs in, full output out.
Shapes: aligned_feat [4, 8, 64, 128, 128] f32, w1/w2 [64, 64, 3, 3],
b1/b2 [64], wf [64, 512, 1, 1], bf [64].

Math (BGFusionBlock):
  emb     = conv3x3(x, w2, b2)  per frame
  emb_ref = conv3x3(x, w1, b1)  per frame
  scores[b,t,h,w] = <emb_ref[b,t], sum_j emb[b,j]>_c
  attn    = softmax(scores / 0.5, over t)
  out     = leaky_relu(conv1x1(aligned_feat * attn, wf, bf), 0.1)

Device strategy:
  * Each of the 8 cores owns 16 output rows (H/8); halo rows + zero padding
    are baked into its input slab host-side, so cores are fully independent
    (no collectives).
  * emb_sum = conv3x3(sum_t x, w2) + T*b2 by linearity -> the w2 conv runs
    on the pre-summed frames (sum over t is done host-side, it is free
    relative to device roofline).
  * b1 drops out: <b1, emb_sum> is constant over t, softmax ignores it.
  * 3x3 convs run as 6 K=128/M=128 bf16 matmuls per 2 output rows: the
    input is stored twice on 128 partitions (channels of row r on
    partitions 0:64, channels of row r+1 on 64:128) and each PSUM tile
    packs [even-row channels | odd-row channels].
  * scores = sum_c emb_ref * emb_sum via a one-hot "ones" matmul that also
    folds the 1/T temperature (x2) and routes frame t / row-parity s to
    PSUM partition t + 8s; softmax over t is a partition tree on 16 rows.
  * attn is re-broadcast across the 64 channel partitions with a DRAM
    bounce DMA (partition-stride-0 read), then weighted = x * attn on DVE
    and the 1x1 conv is 4 K=128 bf16 matmuls (contraction over t,c = 512).
"""

import os
import sys

sys.path.insert(0, "/opt/trn_rl_repo")

import numpy as np
import ml_dtypes

BF16 = ml_dtypes.bfloat16

B, T, C, H, W = 4, 8, 64, 128, 128
NCORES = 8
RPC = H // NCORES          # output rows per core: 16
SLAB = RPC + 2             # input rows per core incl. halo: 18
WP = W + 2                 # padded width: 130
NPAIR = RPC // 2           # row pairs per core: 8
NG = 2                     # pair-groups per core (4 pairs each -> N=512)


def _build_program():
    import concourse.bass as bass
    import concourse.tile as tile
    from concourse import bacc, mybir

    dt = mybir.dt
    nc = bacc.Bacc()

    x2 = nc.declare_dram_parameter("x2", [128, B, T, 9, 2, WP], dt.float16, False)
    xs2 = nc.declare_dram_parameter("xs2", [128, B, 9, 2, WP], dt.float16, False)
    wc1 = nc.declare_dram_parameter("wc1", [3, 2, 128, 128], dt.float16, False)
    wc2 = nc.declare_dram_parameter("wc2", [3, 2, 128, 128], dt.float16, False)
    won = nc.declare_dram_parameter("won", [128, 2], dt.float16, False)
    wf2 = nc.declare_dram_parameter("wf2", [4, 128, 64], dt.float16, False)
    b2s8 = nc.declare_dram_parameter("b2s8", [128, 1], dt.float32, False)
    bfv = nc.declare_dram_parameter("bfv", [64, 1], dt.float32, False)
    out = nc.declare_dram_parameter("out", [B, C, 8, 2, W], dt.float32, True)

    # DRAM bounce buffer for the attention partition-broadcast.
    # layout [b, q=t%4, s, t1=t//4, g, pix512]
    attn_dram = nc.dram_tensor("attn_dram", [B, 4, 2, 2, NG, 512], dt.float16)

    with tile.TileContext(nc) as tc:
        with (
            tc.tile_pool(name="consts", bufs=1) as consts,
            tc.tile_pool(name="xpool", bufs=2) as xpool,
            tc.tile_pool(name="spool", bufs=1) as spool,
            tc.tile_pool(name="ppool", bufs=3) as ppool,
            tc.tile_pool(name="w2pool", bufs=2) as w2pool,
            tc.tile_pool(name="apool", bufs=2) as apool,
            tc.tile_pool(name="smx", bufs=1) as smx,
            tc.tile_pool(name="opool", bufs=2) as opool,
            tc.tile_pool(name="epool", bufs=2, space="PSUM") as epool,
            tc.tile_pool(name="scpool", bufs=1, space="PSUM") as scpool,
            tc.tile_pool(name="fpool", bufs=2, space="PSUM") as fpool,
        ):
            # ---- constants ----
            wc1_sb = consts.tile([128, 3, 2, 128], dt.float16)
            wc2_sb = consts.tile([128, 3, 2, 128], dt.float16)
            for dj in range(3):
                for ab in range(2):
                    nc.sync.dma_start(out=wc1_sb[:, dj, ab, :], in_=wc1[dj, ab])
                    nc.sync.dma_start(out=wc2_sb[:, dj, ab, :], in_=wc2[dj, ab])
            won_sb = consts.tile([128, 2], dt.float16)
            nc.sync.dma_start(out=won_sb[:], in_=won[:])
            wf2_sb = consts.tile([128, 4, 64], dt.float16)
            for tp in range(4):
                nc.sync.dma_start(out=wf2_sb[:, tp, :], in_=wf2[tp])
            b2s8_sb = consts.tile([128, 1], dt.float32)
            nc.sync.dma_start(out=b2s8_sb[:], in_=b2s8[:])
            bfv_sb = consts.tile([64, 1], dt.float32)
            nc.sync.dma_start(out=bfv_sb[:], in_=bfv[:])
            c01_sb = consts.tile([64, 1], dt.float32)
            nc.vector.memset(c01_sb[:], 0.1)

            def _fbc(tile_, dims):
                """[P, 1] const tile -> [P, *dims] stride-0 free broadcast."""
                ap = tile_[:]
                return bass.AP(
                    tensor=ap.tensor,
                    offset=ap.offset,
                    ap=[ap.ap[0]] + [[0, d] for d in dims],
                )

            # x_sum slab, viewed as [128, B, pair, parity, WP]
            xs2_sb = consts.tile([128, B, 9, 2, WP], dt.float16)
            for b in range(B):
                nc.sync.dma_start(out=xs2_sb[:, b], in_=xs2[:, b])

            def conv_mms(e_ps, w_sb, src, t_idx, g):
                """6 matmuls accumulating conv pair-group g into e_ps [128,4,128].

                src: [128, 9, 2, WP] view (pair, parity, col) of one frame.
                MM1 reads even rows 8g+2rp (pairs 4g..4g+3), MM2 reads even
                rows 8g+2rp+2 (pairs 4g+1..4g+4).
                """
                for dj in range(3):
                    nc.tensor.matmul(
                        e_ps[:],
                        w_sb[:, dj, 0, :],
                        src[:, 4 * g : 4 * g + 4, 0, dj : dj + 128],
                        start=(dj == 0),
                        stop=False,
                    )
                    nc.tensor.matmul(
                        e_ps[:],
                        w_sb[:, dj, 1, :],
                        src[:, 4 * g + 1 : 4 * g + 5, 0, dj : dj + 128],
                        start=False,
                        stop=(dj == 2),
                    )

            # ---- phase A: emb_sum = conv3x3(x_sum, w2) + T*b2 ----
            s2_sb = spool.tile([128, B, NG, 4, 128], dt.float32)
            for b in range(B):
                for g in range(NG):
                    e_ps = epool.tile([128, 4, 128], dt.float32, tag="E")
                    conv_mms(e_ps, wc2_sb, xs2_sb[:, b], None, g)
                    nc.vector.tensor_add(
                        s2_sb[:, b, g], e_ps[:], _fbc(b2s8_sb, [4, 128])
                    )

            # ---- phase B: per-batch pipeline ----
            for b in range(B):
                x2b = xpool.tile([128, T, 9, 2, WP], dt.float16, tag="X")
                for th in range(0, T, 2):
                    nc.sync.dma_start(
                        out=x2b[:, th : th + 2], in_=x2[:, b, th : th + 2]
                    )

                # scores: frame t -> PSUM quadrant t%4, rows {s}, free half t//4
                sc_g = [
                    scpool.tile(
                        [128, 2, 512], dt.float32, tag=f"SC{g}", name=f"sc{g}"
                    )
                    for g in range(NG)
                ]
                for t in range(T):
                    q, t1 = t % 4, t // 4
                    for g in range(NG):
                        e_ps = epool.tile([128, 4, 128], dt.float32, tag="E")
                        conv_mms(e_ps, wc1_sb, x2b[:, t], t, g)
                        p_sb = ppool.tile([128, 4, 128], dt.float16, tag="P")
                        nc.vector.tensor_mul(p_sb[:], e_ps[:], s2_sb[:, b, g])
                        nc.tensor.matmul(
                            sc_g[g][32 * q : 32 * q + 2, t1],
                            won_sb[:],
                            p_sb[:],
                            start=True,
                            stop=True,
                            tile_position=(0, 32 * q),
                        )

                # softmax over t, per pair-group g.
                # HW rule: two SBUF inputs of a tensor-tensor op must share
                # the base partition, so cross-quadrant reductions always
                # keep one operand in PSUM (sc_g) and accumulate into a
                # base-0 SBUF scratch; exp(s - max) is written back into
                # the PSUM tile in place.
                a_sb = smx.tile([128, NG, 2, 512], dt.float16, tag="attn")
                for g in range(NG):
                    mxq = smx.tile([2, 2, 512], dt.float32, tag="mxq")
                    mx2 = smx.tile([2, 512], dt.float32, tag="mx2")
                    nc.vector.tensor_copy(mxq[:], sc_g[g][0:2])
                    nc.vector.tensor_max(mxq[:], sc_g[g][32:34], mxq[:])
                    nc.vector.tensor_max(mxq[:], sc_g[g][64:66], mxq[:])
                    nc.vector.tensor_max(mxq[:], sc_g[g][96:98], mxq[:])
                    nc.vector.tensor_max(mx2[:], mxq[:, 0], mxq[:, 1])
                    bc2 = bass.AP(
                        tensor=mx2[:].tensor,
                        offset=mx2[:].offset,
                        ap=[mx2[:].ap[0], [0, 2], [1, 512]],
                    )
                    for q in range(4):
                        nc.vector.tensor_sub(
                            sc_g[g][32 * q : 32 * q + 2],
                            sc_g[g][32 * q : 32 * q + 2],
                            bc2,
                        )
                        nc.scalar.activation(
                            sc_g[g][32 * q : 32 * q + 2],
                            sc_g[g][32 * q : 32 * q + 2],
                            mybir.ActivationFunctionType.Exp,
                        )
                    nc.vector.tensor_copy(mxq[:], sc_g[g][0:2])
                    nc.vector.tensor_add(mxq[:], sc_g[g][32:34], mxq[:])
                    nc.vector.tensor_add(mxq[:], sc_g[g][64:66], mxq[:])
                    nc.vector.tensor_add(mxq[:], sc_g[g][96:98], mxq[:])
                    nc.vector.tensor_add(mx2[:], mxq[:, 0], mxq[:, 1])
                    nc.vector.reciprocal(mx2[:], mx2[:])
                    for q in range(4):
                        nc.vector.tensor_mul(
                            a_sb[32 * q : 32 * q + 2, g],
                            sc_g[g][32 * q : 32 * q + 2],
                            bc2,
                        )
                        nc.sync.dma_start(
                            out=attn_dram[b, q, :, :, g, :],
                            in_=a_sb[32 * q : 32 * q + 2, g],
                        )

                # weighted = x * attn (bcast over channels via DRAM bounce)
                w2b = w2pool.tile([128, 4, 8, 2, 128], dt.float16, tag="W2")
                base = attn_dram[:]
                for t in range(T):
                    q, t1 = t % 4, t // 4
                    a_rep = apool.tile([64, 2, 8, 128], dt.float16, tag="arep")
                    src = bass.AP(
                        tensor=base.tensor,
                        offset=base.offset + b * 16384 + q * 4096 + t1 * 1024,
                        ap=[[0, 64], [2048, 2], [128, 8], [1, 128]],
                    )
                    nc.sync.dma_start(out=a_rep[:], in_=src)
                    pp = 64 * (t % 2)
                    for s in range(2):
                        if s == 0:
                            xsl = x2b[0:64, t, 0:8, 1, 1:129]
                        else:
                            xsl = x2b[0:64, t, 1:9, 0, 1:129]
                        nc.vector.tensor_mul(
                            w2b[pp : pp + 64, t // 2, :, s, :],
                            xsl,
                            a_rep[:, s],
                        )

                # 1x1 conv over (t, c) + bias + leaky relu
                ob = opool.tile([64, 8, 2, 128], dt.float32, tag="OB")
                for n in range(4):
                    f_ps = fpool.tile([64, 2, 2, 128], dt.float32, tag="F")
                    for tp in range(4):
                        nc.tensor.matmul(
                            f_ps[:],
                            wf2_sb[:, tp, :],
                            w2b[:, tp, 2 * n : 2 * n + 2],
                            start=(tp == 0),
                            stop=(tp == 3),
                        )
                    fb = opool.tile([64, 2, 2, 128], dt.float32, tag="FB")
                    xs = opool.tile([64, 2, 2, 128], dt.float32, tag="XS")
                    # TensorScalar encodings only carry one sync-wait slot
                    # (walrus setupSyncWait), so stick to tensor_tensor with
                    # stride-0 free-dim broadcast APs for bias/leaky.
                    nc.vector.tensor_add(fb[:], f_ps[:], _fbc(bfv_sb, [2, 2, 128]))
                    nc.vector.tensor_mul(xs[:], fb[:], _fbc(c01_sb, [2, 2, 128]))
                    nc.vector.tensor_max(ob[:, 2 * n : 2 * n + 2], fb[:], xs[:])
                nc.sync.dma_start(out=out[b], in_=ob[:])

    nc.finalize()
    return nc


def _host_prep(aligned_feat, w1, b1, w2, b2, wf, bf):
    """Build per-core input maps (sharding + padding + weight packing)."""
    x = np.ascontiguousarray(aligned_feat, dtype=np.float32)
    # zero-pad: rows 131 (1 top + 2 bottom margin), cols 130
    xpad = np.zeros((B, T, C, H + 3, W + 2), dtype=np.float32)
    xpad[:, :, :, 1 : H + 1, 1 : W + 1] = x
    xspad = xpad.sum(axis=1)  # [B, C, 131, 130]

    def pack_conv(w):
        # lhsT pair matrices: A_dj = [[W0T, 0], [W1T, W0T]], B_dj = [[W2T, W1T], [0, W2T]]
        wt = w.astype(np.float32)
        z = np.zeros((64, 64), np.float32)
        mats = np.empty((3, 2, 128, 128), np.float32)
        for dj in range(3):
            w0, w1_, w2_ = (wt[:, :, i, dj].T for i in range(3))
            mats[dj, 0] = np.block([[w0, z], [w1_, w0]])
            mats[dj, 1] = np.block([[w2_, w1_], [z, w2_]])
        return mats.astype(np.float16)

    wc1_np = pack_conv(np.asarray(w1))
    wc2_np = pack_conv(np.asarray(w2))

    won_np = np.zeros((128, 2), np.float32)
    won_np[0:64, 0] = 2.0        # fold 1/temperature
    won_np[64:128, 1] = 2.0
    won_np = won_np.astype(np.float16)

    wf_r = np.asarray(wf, dtype=np.float32).reshape(64, T, 64)  # [o, t, c]
    wf2_np = np.stack(
        [
            np.concatenate([wf_r[:, 2 * tp].T, wf_r[:, 2 * tp + 1].T], axis=0)
            for tp in range(4)
        ]
    ).astype(np.float16)

    b2s8_np = (np.tile(np.asarray(b2, np.float32), 2) * T).reshape(128, 1)
    bfv_np = np.asarray(bf, np.float32).reshape(64, 1)

    consts = {
        "wc1": wc1_np,
        "wc2": wc2_np,
        "won": won_np,
        "wf2": wf2_np,
        "b2s8": b2s8_np,
        "bfv": bfv_np,
    }

    in_maps = []
    for k in range(NCORES):
        r0 = RPC * k
        top = xpad[:, :, :, r0 : r0 + SLAB, :]          # [B,T,C,18,130]
        bot = xpad[:, :, :, r0 + 1 : r0 + SLAB + 1, :]
        x2_np = (
            np.concatenate(
                [top.transpose(2, 0, 1, 3, 4), bot.transpose(2, 0, 1, 3, 4)], axis=0
            )
            .astype(np.float16)
            .reshape(128, B, T, 9, 2, WP)
        )
        tops = xspad[:, :, r0 : r0 + SLAB, :]
        bots = xspad[:, :, r0 + 1 : r0 + SLAB + 1, :]
        xs2_np = (
            np.concatenate(
                [tops.transpose(1, 0, 2, 3), bots.transpose(1, 0, 2, 3)], axis=0
            )
            .astype(np.float16)
            .reshape(128, B, 9, 2, WP)
        )
        in_maps.append({"x2": x2_np, "xs2": xs2_np, **consts})
    return in_maps


_NC_CACHE = {}


def _get_program():
    if "nc" not in _NC_CACHE:
        _NC_CACHE["nc"] = _build_program()
    return _NC_CACHE["nc"]


def kernel(aligned_feat, w1, b1, w2, b2, wf, bf):
    from concourse.bass_utils import run_bass_kernel_spmd

    nc = _get_program()
    in_maps = _host_prep(aligned_feat, w1, b1, w2, b2, wf, bf)
    trace = bool(int(os.environ.get("BG_TRACE", "0")))
    res = run_bass_kernel_spmd(
        nc, in_maps, list(range(NCORES)), trace=trace
    )
    _NC_CACHE["last_result"] = res
    full = np.empty((B, C, H, W), dtype=np.float32)
    for k in range(NCORES):
        full[:, :, RPC * k : RPC * (k + 1), :] = res.results[k]["out"].reshape(
            B, C, RPC, W
        )
    return full
